# revision 4
# baseline (speedup 1.0000x reference)
"""Multi-head attention (B=4, S=2048, D=2048, H=16 heads, R=128) on 8 Trainium2
NeuronCores, tensor-parallel over heads (2 heads per core), with a final
AllReduce over the W_O row-contraction.

Numerics: the softmax path (Q/K projections and Q.K^T scores) runs as 3-pass
bf16 hi/lo ("f32x3") matmuls so scores carry ~f32 precision — the scores have
std ~2048 so the softmax is extremely sharp and bf16-only scores would flip
near-tie argmaxes.  The value path (V projection) is also 3-pass by default;
probs/V/attn/W_O matmuls run in native f32 (exact).
"""

import os
import sys
import types

sys.path.insert(0, "/opt/trn_rl_repo")

import numpy as np
import ml_dtypes

# ─────────────────────────────── constants ───────────────────────────────
B, S, D = 4, 2048, 2048
H, R = 16, 128
N_CORES = 8
HPC = H // N_CORES          # heads per core = 2
RW = HPC * R                # per-core projection width = 256
T = B * S                   # 8192 tokens
DC = D // 128               # 16 contraction chunks
SCALE = 1.0 / (R ** 0.5)

# config knobs (override via env for experiments)
V_PASSES = int(os.environ.get("K_V_PASSES", "3"))        # 1 or 3
QK_PASSES = int(os.environ.get("K_QK_PASSES", "3"))      # 3 (keep)
ATTNV_DTYPE = os.environ.get("K_ATTNV", "f32")           # f32 | f32r | bf16
X_BUFS = int(os.environ.get("K_X_BUFS", "18"))  # per tag (xh, xl)

LAST_EXEC_TIME_NS = [None]


# ───────────────────────── harness glue (inlined) ─────────────────────────
def _install_ntff_hook():
    """Wire the missing antenv.axon_hooks module so trace=True can profile."""
    try:
        import antenv.axon_hooks  # noqa: F401
        return
    except ImportError:
        pass
    try:
        import antenv
        from trn_agent_boot.trn_boot import _ntff_profile_via_ctypes
    except ImportError:
        return
    mod = types.ModuleType("antenv.axon_hooks")
    _hook = [None]
    mod.set_axon_ntff_profile_hook = lambda h: _hook.__setitem__(0, h)
    mod.get_axon_ntff_profile_hook = lambda: _hook[0]
    antenv.axon_hooks = mod
    sys.modules["antenv.axon_hooks"] = mod
    try:
        mod.set_axon_ntff_profile_hook(
            _ntff_profile_via_ctypes("/opt/axon/libaxon_pjrt.so")
        )
    except Exception:
        pass


def _split_excess_waits(nc, max_waits=1):
    """walrus on this toolchain rejects >1 sem-wait per instruction; hoist
    the excess onto preceding same-engine NoOps."""
    from concourse import mybir

    for fn in nc.m.functions:
        for bb in fn.blocks:
            insts = list(bb.instructions)
            out = []
            changed = False
            for inst in insts:
                si = inst.sync_info
                if si is not None and si.on_wait and len(si.on_wait) > max_waits:
                    waits = list(si.on_wait)
                    chunks = [
                        waits[i : i + max_waits]
                        for i in range(0, len(waits), max_waits)
                    ]
                    for ci, chunk in enumerate(chunks[:-1]):
                        out.append(
                            mybir.InstNoOp(
                                name=f"{inst.name}-ws{ci}",
                                engine=inst.engine,
                                ins=[],
                                outs=[],
                                sync_info=mybir.SyncInfo(
                                    on_wait=list(chunk), on_update=[]
                                ),
                                text_hint="waitsplit",
                            )
                        )
                    si.on_wait = list(chunks[-1])
                    changed = True
                out.append(inst)
            if changed:
                try:
                    bb.instructions = out
                except Exception:
                    bb.instructions.clear()
                    for i in out:
                        bb.instructions.append(i)


# ───────────────────────────── device kernel ─────────────────────────────
def _build_nc():
    from contextlib import ExitStack

    import concourse.bass as bass
    import concourse.tile as tile
    from concourse import mybir
    from concourse.masks import make_identity

    f32 = mybir.dt.float32
    bf16 = mybir.dt.bfloat16
    AX = mybir.AxisListType
    EXP = mybir.ActivationFunctionType.Exp

    nc = bass.Bass(
        "TRN2", target_bir_lowering=False, debug=False, num_devices=N_CORES
    )

    xh_ap = nc.dram_tensor("xh", [D, T], bf16, kind="ExternalInput").ap()
    xl_ap = nc.dram_tensor("xl", [D, T], bf16, kind="ExternalInput").ap()
    w_ap = {
        n: nc.dram_tensor(n, [D, RW], bf16, kind="ExternalInput").ap()
        for n in ("wqh", "wql", "wkh", "wkl", "wvh", "wvl")
    }
    wo_ap = nc.dram_tensor("wo", [RW, R], f32, kind="ExternalInput").ap()
    out_ap = nc.dram_tensor("out", [T, R], f32, kind="ExternalOutput").ap()
    ar_in = nc.dram_tensor("ar_in", [T, R], f32)
    ar_out = nc.dram_tensor("ar_out", [T, R], f32, addr_space="Shared")

    attn_f32 = ATTNV_DTYPE in ("f32", "f32r")
    p_dt = f32 if attn_f32 else bf16
    if ATTNV_DTYPE == "f32r":
        p_dt = mybir.dt.float32r
    v_dt = p_dt if attn_f32 else bf16

    with tile.TileContext(nc) as tc, ExitStack() as ctx:
        P = lambda **kw: ctx.enter_context(tc.tile_pool(**kw))
        const = P(name="const", bufs=1)
        x_pool = P(name="x", bufs=X_BUFS)
        qkv_pool = P(name="qkv", bufs=1)
        s_pool = P(name="s", bufs=2)
        p_pool = P(name="p", bufs=2)
        pt_pool = P(name="pt", bufs=2)
        ot_pool = P(name="ot", bufs=3)
        tmp_pool = P(name="tmp", bufs=2)
        stats = P(name="stats", bufs=4)
        ps = P(name="ps", bufs=1, space="PSUM")  # bufs set per tile() call

        # resident weights: [128, DC*RW], column block dc holds W[dc*128:(dc+1)*128, :]
        w_sb = {}
        for n in ("wqh", "wql", "wkh", "wkl", "wvh", "wvl"):
            if V_PASSES == 1 and n == "wvl":
                continue
            t = const.tile([128, DC * RW], bf16, tag=n, name=n)
            for dc in range(DC):
                nc.sync.dma_start(
                    t[:, dc * RW : (dc + 1) * RW],
                    w_ap[n][dc * 128 : (dc + 1) * 128, :],
                )
            w_sb[n] = t
        wo_sb = const.tile([128, HPC * R], f32, tag="wo", name="wo_sb")
        for rh in range(HPC):
            nc.sync.dma_start(
                wo_sb[:, rh * R : (rh + 1) * R],
                wo_ap[rh * 128 : (rh + 1) * 128, :],
            )
        ident = const.tile([128, 128], p_dt if attn_f32 else bf16, tag="ident", name="ident")
        make_identity(nc, ident[:])

        for b in range(B):
            tb0 = b * S

            # ── phase A: projections for this batch ──
            # Q^T/K^T hi+lo: [128 r, 2048 t] per head-half
            qt = {
                (m, p): [
                    qkv_pool.tile([128, S], bf16, tag=f"{m}{p}{rh}", name=f"{m}{p}{rh}")
                    for rh in range(HPC)
                ]
                for m in ("q", "k")
                for p in ("h", "l")
            }
            v_sb = qkv_pool.tile([128, DC * RW], v_dt, tag="v", name="v_sb")

            for tg in range(4):
                t0 = tb0 + tg * 512
                xh_t, xl_t = [], []
                for dc in range(DC):
                    th = x_pool.tile([128, 512], bf16, tag="xh", name="xh_t")
                    nc.sync.dma_start(
                        th[:], xh_ap[dc * 128 : (dc + 1) * 128, t0 : t0 + 512]
                    )
                    xh_t.append(th)
                    tl = x_pool.tile([128, 512], bf16, tag="xl", name="xl_t")
                    nc.sync.dma_start(
                        tl[:], xl_ap[dc * 128 : (dc + 1) * 128, t0 : t0 + 512]
                    )
                    xl_t.append(tl)

                # Q^T, K^T
                for m, wh, wl in (("q", "wqh", "wql"), ("k", "wkh", "wkl")):
                    for rh in range(HPC):
                        psp = ps.tile([128, 512], f32, tag="pa", bufs=2, name="ps_proj")
                        passes = [
                            (w_sb[wh], xh_t),
                            (w_sb[wl], xh_t),
                            (w_sb[wh], xl_t),
                        ][:QK_PASSES]
                        n_mm = len(passes) * DC
                        i = 0
                        for wt, xt in passes:
                            for dc in range(DC):
                                nc.tensor.matmul(
                                    psp[:],
                                    lhsT=wt[
                                        :,
                                        dc * RW + rh * 128 : dc * RW + rh * 128 + 128,
                                    ],
                                    rhs=xt[dc][:],
                                    start=(i == 0),
                                    stop=(i == n_mm - 1),
                                )
                                i += 1
                        dst_h = qt[(m, "h")][rh][:, tg * 512 : (tg + 1) * 512]
                        dst_l = qt[(m, "l")][rh][:, tg * 512 : (tg + 1) * 512]
                        nc.scalar.copy(dst_h, psp[:])
                        nc.vector.tensor_sub(dst_l, psp[:], dst_h)

                # V (natural layout [t, r])
                for tb in range(4):
                    psv = ps.tile([128, RW], f32, tag="pa", bufs=2, name="ps_vproj")
                    vpasses = [
                        (xh_t, "wvh"),
                        (xl_t, "wvh"),
                        (xh_t, "wvl"),
                    ][:V_PASSES]
                    n_mm = len(vpasses) * DC
                    i = 0
                    for xt, wn in vpasses:
                        for dc in range(DC):
                            nc.tensor.matmul(
                                psv[:],
                                lhsT=xt[dc][:, tb * 128 : (tb + 1) * 128],
                                rhs=w_sb[wn][:, dc * RW : (dc + 1) * RW],
                                start=(i == 0),
                                stop=(i == n_mm - 1),
                            )
                            i += 1
                    tbi = tg * 4 + tb
                    nc.scalar.copy(v_sb[:, tbi * RW : (tbi + 1) * RW], psv[:])

            # ── phase B: attention, heads interleaved per q-block ──
            for qb in range(16):
                o2s = []
                for h in range(HPC):
                    q0 = qb * 128
                    # scores [128 q, 2048 k], f32x3 accumulation; each
                    # 512-k psum tile is copied to SBUF as soon as its 3-pass
                    # accumulation completes, freeing the bank.
                    spasses = [
                        (qt[("q", "h")], qt[("k", "h")]),
                        (qt[("q", "l")], qt[("k", "h")]),
                        (qt[("q", "h")], qt[("k", "l")]),
                    ][:QK_PASSES]
                    np_ = len(spasses)
                    s_sb = s_pool.tile([128, S], f32, tag="s", name="s_sb")
                    for ktp in range(2):  # pairs of k-tiles
                        pss = [
                            ps.tile([128, 512], f32, tag="s", bufs=3, name="ps_s")
                            for _ in range(2)
                        ]
                        for pi, (qsrc, ksrc) in enumerate(spasses):
                            for kj in range(2):
                                kt = ktp * 2 + kj
                                nc.tensor.matmul(
                                    pss[kj][:],
                                    lhsT=qsrc[h][:, q0 : q0 + 128],
                                    rhs=ksrc[h][:, kt * 512 : (kt + 1) * 512],
                                    start=(pi == 0),
                                    stop=(pi == np_ - 1),
                                )
                        for kj in range(2):
                            kt = ktp * 2 + kj
                            nc.scalar.copy(
                                s_sb[:, kt * 512 : (kt + 1) * 512], pss[kj][:]
                            )

                    negmax = stats.tile([128, 1], f32, tag="negmax", name="negmax")
                    nc.vector.reduce_max(negmax[:], s_sb[:], axis=AX.X, negate=True)
                    bias = stats.tile([128, 1], f32, tag="bias", name="bias")
                    nc.scalar.mul(bias[:], negmax[:], SCALE)
                    p_t = p_pool.tile([128, S], p_dt, tag="p", name="p_t")
                    ssum = stats.tile([128, 1], f32, tag="ssum", name="ssum")
                    nc.scalar.activation(
                        p_t[:], s_sb[:], EXP, bias=bias[:], scale=SCALE,
                        accum_out=ssum[:],
                    )
                    rc = stats.tile([128, 1], f32, tag=f"recip{h}", name="rc")
                    nc.vector.reciprocal(rc[:], ssum[:])

                    # transpose P: [128 q, 2048 k] -> pt_sb [128 k, 16kc*128 q]
                    pt_sb = pt_pool.tile([128, DC * 128], p_dt, tag="pt", name="pt_sb")
                    for g in range(4):
                        pst = ps.tile([128, 512], p_dt, tag="pt", bufs=2, name="ps_pt")
                        for j in range(4):
                            kc = g * 4 + j
                            nc.tensor.transpose(
                                pst[:, j * 128 : (j + 1) * 128],
                                p_t[:, kc * 128 : (kc + 1) * 128],
                                ident[:],
                            )
                        nc.vector.tensor_copy(
                            pt_sb[:, g * 512 : (g + 1) * 512], pst[:]
                        )

                    # attn = P @ V, accumulated transposed: O^T [128 r, 128 q]
                    ps_ot = ps.tile([128, 128], f32, tag="ot", bufs=1, name="ps_ot")
                    for kc in range(DC):
                        nc.tensor.matmul(
                            ps_ot[:],
                            lhsT=v_sb[
                                :, kc * RW + h * 128 : kc * RW + h * 128 + 128
                            ],
                            rhs=pt_sb[:, kc * 128 : (kc + 1) * 128],
                            start=(kc == 0),
                            stop=(kc == DC - 1),
                        )
                    ot_sb = ot_pool.tile([128, 128], f32, tag="ot", name="ot_sb")
                    nc.scalar.copy(ot_sb[:], ps_ot[:])

                    # out2 [128 q, 128] = O^T.T @ Wo_h  (native f32)
                    ps_o2 = ps.tile([128, 512], f32, tag="pa", bufs=2, name="ps_o2")
                    nc.tensor.matmul(
                        ps_o2[:, 0:128],
                        lhsT=ot_sb[:],
                        rhs=wo_sb[:, h * R : (h + 1) * R],
                        start=True,
                        stop=True,
                    )
                    tmp = tmp_pool.tile([128, 128], f32, tag=f"o2s{h}", name="tmp")
                    nc.scalar.mul(tmp[:], ps_o2[:, 0:128], rc[:])
                    o2s.append(tmp)
                res = tmp_pool.tile([128, 128], f32, tag="res", name="res")
                nc.vector.tensor_add(res[:], o2s[0][:], o2s[1][:])
                nc.sync.dma_start(
                    ar_in.ap()[tb0 + qb * 128 : tb0 + (qb + 1) * 128, :],
                    res[:],
                )

        nc.gpsimd.collective_compute(
            "AllReduce",
            mybir.AluOpType.add,
            replica_groups=[list(range(N_CORES))],
            ins=[ar_in.ap()[:]],
            outs=[ar_out.ap()[:]],
        )
        nc.sync.dma_start(out_ap[:], ar_out.ap()[:])

    return nc


# ─────────────────────────────── host entry ───────────────────────────────
def _split_hi_lo(a):
    hi = a.astype(ml_dtypes.bfloat16)
    lo = (a - hi.astype(np.float32)).astype(ml_dtypes.bfloat16)
    return hi, lo


def kernel(X, mask, W_Q, W_K, W_V, W_O):
    _install_ntff_hook()
    from concourse.bass_utils import run_bass_kernel_spmd

    X2 = np.ascontiguousarray(
        np.asarray(X, dtype=np.float32).reshape(T, D).T
    )  # [D, T]
    xh, xl = _split_hi_lo(X2)
    W_Q = np.asarray(W_Q, np.float32)
    W_K = np.asarray(W_K, np.float32)
    W_V = np.asarray(W_V, np.float32)
    W_O = np.asarray(W_O, np.float32)

    in_maps = []
    for c in range(N_CORES):
        cols = slice(c * RW, (c + 1) * RW)
        wqh, wql = _split_hi_lo(W_Q[:, cols])
        wkh, wkl = _split_hi_lo(W_K[:, cols])
        wvh, wvl = _split_hi_lo(W_V[:, cols])
        in_maps.append(
            {
                "xh": xh,
                "xl": xl,
                "wqh": wqh,
                "wql": wql,
                "wkh": wkh,
                "wkl": wkl,
                "wvh": wvh,
                "wvl": wvl,
                "wo": np.ascontiguousarray(W_O[cols, :]),
            }
        )

    nc = _build_nc()
    _split_excess_waits(nc)
    trace = bool(int(os.environ.get("KERNEL_TRACE", "0")))
    res = run_bass_kernel_spmd(
        nc, in_maps, list(range(N_CORES)), trace=trace
    )
    LAST_EXEC_TIME_NS[0] = res.exec_time_ns
    out = np.asarray(res.results[0]["out"], dtype=np.float32)
    return out.reshape(B, S, R)
